# revision 47
# baseline (speedup 1.0000x reference)
"""Dynamic lightweight convolution TRN2 kernel.

out[b,l,d] = (1/K) * sum_k softmax_k(x[b,l+K-1,:] @ W + bias)[k, d%H] * x[b,l+k,d]

B=8, S=2048, D=1024, K=7, H=16, L=S-K+1=2042.
Sharding: data-parallel over batch, one batch element per NeuronCore (8 cores).

All sequence-major work stays in the transposed [d, s] layout end-to-end.
The host supplies x already transposed and bf16-cast (mirroring the
host-packed constants blob), and takes the output back in [d, l] bf16, so
the device program has NO transposes:
  1. DMA xtb[p, c, s] = x^T bf16 straight into SBUF.
  2. logits = W^T @ xtb on PE (fp32 PSUM accum over 8 d-chunks); E =
     exp(logits + bias) on ACT; selector matmul gives K*sum_k E (PE);
     Rinv ~ 1/that (DVE custom fast-reciprocal); en = E * Rinv (DVE).
  3. m[p, k, l] = en[16k + p%16, l+6] via 0/1 selector matmuls (PE) +
     ACT PSUM->SBUF bf16 copies, per conv block, emitted 2 blocks ahead
     of the conv so no engine ever starves waiting on weights.
  4. conv products p_k = m_k * x_(+k): DVE gets batched ops (one per
     k over a c-range, m broadcast over c via stride-0 dim, 2x bf16
     mode); GPSIMD (Pool) takes the (c, k) units in POOL_KS as per-c ops
     ordered c0->c3 so PE's per-c psum streams unblock progressively.
     The 7-way k-sum runs on PE as accumulating identity matmuls into
     PSUM (fp32), except DVE_TREE (block, c) units summed on DVE.
  5. ACT copies conv PSUM -> acc bf16; DMA out in [d, l] layout; host
     transposes back to [l, d] and upcasts to f32.
"""

import numpy as np
import ml_dtypes
from contextlib import ExitStack

import concourse.bacc as bacc
import concourse.tile as tile
from concourse import mybir
from concourse import bass_utils

K = 7
H = 16
B, S, D = 8, 2048, 1024
L = S - K + 1  # 2042
C = D // 128  # 8 d-chunks
SB = 512
KH = K * H  # 112

F32 = mybir.dt.float32
BF16 = mybir.dt.bfloat16

# front sub-ranges (logits/exp/denominator granularity)
FRONTS = [(0, 256), (256, 512), (512, 1024), (1024, 1536), (1536, 2048)]
# conv/mrep block boundaries; block j needs en/xtb cols < CVB[j+1] + 6
CVB = [0, 250, 506, 1018, 1530, 1914, 2042]
NCV = len(CVB) - 1

# Pool (gpsimd) product units: per-c k-tuples, processed c-ascending;
# plus one [c4,c5] pair op for k=6. DVE covers the complement with
# batched ops: k0..3 over c0..7, k4/k5 over c3..7, k6 over c6..7.
POOL_KS = {0: (4, 5, 6), 1: (4, 5, 6), 2: (4, 5, 6), 3: (6,)}
# blocks 0..3 also put (k5, c3) on pool to fill its early-finish idle
POOL_K5C3_BLOCKS = {0, 1, 2, 3}
# (block, c) units whose k-sum runs as a DVE add tree instead of PE psum
# accumulation (c=7 products are all DVE-made, so the tree is local).
DVE_TREE = {(2, 7), (3, 7), (4, 7)}

# byte offsets (per partition) inside the packed constants blob; split so
# the first DMA carries what front(0) needs (bias/W/selsum) and the
# second the rest (identity for PE streams, selk for mrep).
_OFF_BIAS = 0       # [112, 1] f32
_OFF_WT = 4         # [128, 8, 112] bf16
_OFF_SELSUM = 1796  # [112, 112] bf16
_SPLIT = 2020       # first-DMA byte count (505 f32 cols)
_OFF_IDENTB = 2020  # [128, 128] bf16
_OFF_SELK = 2276    # [112, 896] bf16
_CONST_BYTES = 4068  # 1017 f32 columns


def _host_constants(W, b):
    """Pack bias/W/selsum/identb/selk into one [128, 1017] f32 blob."""
    buf = np.zeros((128, _CONST_BYTES), np.uint8)

    def put(off, arr):
        by = np.ascontiguousarray(arr).view(np.uint8).reshape(arr.shape[0], -1)
        buf[: arr.shape[0], off : off + by.shape[1]] = by

    put(_OFF_BIAS, np.asarray(b, np.float32).reshape(KH, 1))
    # W [D, KH] -> [128, C, KH] chunks (d = c*128 + p)
    wt = np.asarray(W, np.float32).astype(ml_dtypes.bfloat16)
    wt = wt.reshape(C, 128, KH).transpose(1, 0, 2).reshape(128, C * KH)
    put(_OFF_WT, np.ascontiguousarray(wt))
    h = np.arange(KH) % H
    selsum = ((h[:, None] == h[None, :]) * float(K)).astype(ml_dtypes.bfloat16)
    put(_OFF_SELSUM, selsum)
    put(_OFF_IDENTB, np.eye(128).astype(ml_dtypes.bfloat16))
    selk = np.zeros((KH, K * 128), dtype=ml_dtypes.bfloat16)
    for k in range(K):
        for p in range(128):
            selk[16 * k + p % 16, k * 128 + p] = 1.0
    put(_OFF_SELK, selk)
    return buf.view(np.float32)


def build_program():
    nc = bacc.Bacc(
        "TRN2", target_bir_lowering=False, debug=False, enable_asserts=True
    )

    xt_d = nc.dram_tensor("xt", [128, C * S], BF16, kind="ExternalInput").ap()
    consts_d = nc.dram_tensor(
        "consts", [128, _CONST_BYTES // 4], F32, kind="ExternalInput"
    ).ap()
    out_d = nc.dram_tensor("out", [128, C * L], BF16, kind="ExternalOutput").ap()

    xt_v = xt_d.rearrange("p (c s) -> p c s", c=C)
    out_v = out_d.rearrange("p (c l) -> p c l", c=C)

    with tile.TileContext(nc) as tc, ExitStack() as ctx:
        singles = ctx.enter_context(tc.tile_pool(name="singles", bufs=1))
        m_pool = ctx.enter_context(tc.tile_pool(name="mw", bufs=3))
        acc_pool = ctx.enter_context(tc.tile_pool(name="acc", bufs=3))
        p8_pool = ctx.enter_context(tc.tile_pool(name="p8", bufs=5))
        p5_pool = ctx.enter_context(tc.tile_pool(name="p5", bufs=4))
        p2_pool = ctx.enter_context(tc.tile_pool(name="p2", bufs=8))
        p1_pool = ctx.enter_context(tc.tile_pool(name="p1", bufs=10))

        p_log = ctx.enter_context(tc.tile_pool(name="plog", bufs=1, space="PSUM"))
        p_mk = ctx.enter_context(tc.tile_pool(name="pmk", bufs=2, space="PSUM"))
        p_cv = ctx.enter_context(tc.tile_pool(name="pcv", bufs=5, space="PSUM"))

        # ---- constants (split: front deps first) + x blocks ----
        cblob = singles.tile([128, _CONST_BYTES // 4], F32)
        nc.sync.dma_start(
            out=cblob[:, : _SPLIT // 4], in_=consts_d[:, : _SPLIT // 4]
        )
        cbytes = cblob.bitcast(mybir.dt.uint8)

        def cview(off, nbytes, dt, rows=128):
            return cbytes[:rows, off : off + nbytes].bitcast(dt)

        bias_t = cview(_OFF_BIAS, 4, F32, rows=KH)
        wt = cview(_OFF_WT, 1792, BF16).rearrange("p (c n) -> p c n", c=C)
        selsum_t = cview(_OFF_SELSUM, 224, BF16, rows=KH)
        identb_t = cview(_OFF_IDENTB, 256, BF16)
        selk_t = cview(_OFF_SELK, 1792, BF16, rows=KH).rearrange(
            "c (k p) -> c k p", k=K
        )

        # ---- persistent tensors ----
        xtb = singles.tile([128, C, S], BF16)  # x^T bf16
        e_full = singles.tile([KH, S], BF16)  # exp(logits + b)
        rinv = singles.tile([KH, S], F32)  # 1 / (K * sum_k E)
        rinv_b = singles.tile([KH, S], BF16)  # bf16 copy (2x en-mult mode)
        en = singles.tile([KH, S], BF16)  # normalized kernel weights

        def load(s0, s1, eng=None):
            (eng or nc.sync).dma_start(
                out=xtb[:, :, s0:s1], in_=xt_v[:, :, s0:s1]
            )

        # first block load on the ACT hwdge queue so its descriptor-gen
        # setup overlaps the consts DMA's (transfers still serialize)
        load(0, 256, eng=nc.scalar)
        nc.sync.dma_start(
            out=cblob[:, _SPLIT // 4 :], in_=consts_d[:, _SPLIT // 4 :]
        )
        # GPSIMD ucode warmup: force the TT library load before real work
        warm = singles.tile([1, 8], BF16)
        nc.gpsimd.tensor_mul(warm, selsum_t[:1, :8], selsum_t[:1, :8])
        # Warmups run on a memset tile so they have NO DMA dependency and
        # fill the otherwise-idle window while x/consts DMAs are in flight:
        # ACT loads its Exp table, PE ramps its clock out of the cold
        # pstate so the first logits matmul runs at full speed.
        wtile = singles.tile([128, 128], BF16)
        nc.vector.memset(wtile, 1.0)
        warm_e = singles.tile([KH, 1], F32)
        nc.scalar.activation(
            warm_e, wtile[:KH, :1], mybir.ActivationFunctionType.Exp
        )
        wpsum = p_log.tile([KH, SB], F32, tag="plog")
        for _ in range(26):
            nc.tensor.matmul(
                wpsum[:, :128], wtile[:, :KH], wtile, start=True, stop=True
            )
        load(256, 512)
        load(512, 1024)
        load(1024, 1536)
        load(1536, 2048)

        def front(fi):
            """logits -> exp -> softmax denom -> normalized weights en."""
            s0, s1 = FRONTS[fi]
            sl = slice(s0, s1)
            ns = s1 - s0
            plog = p_log.tile([KH, SB], F32, tag="plog")
            for c in range(C):
                nc.tensor.matmul(
                    plog[:, :ns],
                    wt[:, c, :],
                    xtb[:, c, sl],
                    start=(c == 0),
                    stop=(c == C - 1),
                )
            nc.scalar.activation(
                e_full[:, sl],
                plog[:, :ns],
                mybir.ActivationFunctionType.Exp,
                bias=bias_t,
                scale=1.0,
            )
            psum = p_log.tile([KH, SB], F32, tag="plog")
            nc.tensor.matmul(
                psum[:, :ns], selsum_t, e_full[:, sl], start=True, stop=True
            )
            nc.vector.reciprocal_approx_fast(rinv[:, sl], psum[:, :ns])
            nc.scalar.copy(rinv_b[:, sl], rinv[:, sl])
            nc.vector.tensor_mul(en[:, sl], e_full[:, sl], rinv_b[:, sl])

        m_tiles = {}
        acc_tiles = {}

        def mrep(j):
            """m_j[p, k, i] = en[16k + p%16, CVB[j] + i + K - 1]."""
            l0, l1 = CVB[j], CVB[j + 1]
            nl = l1 - l0
            mt = m_pool.tile([128, K, SB], BF16, tag="mw")
            m_tiles[j] = mt
            # copy order serves consumers' first needs: DVE's k0 batch,
            # then pool's k6 pair / k4 / k5, then the remaining DVE batches
            for k in (0, 6, 4, 5, 1, 2, 3):
                pmk = p_mk.tile([128, SB], F32, tag="pmk")
                nc.tensor.matmul(
                    pmk[:, :nl],
                    selk_t[:, k, :],
                    en[:, l0 + K - 1 : l0 + K - 1 + nl],
                    start=True,
                    stop=True,
                )
                nc.scalar.copy(mt[:, k, :nl], pmk[:, :nl])

        def conv(j):
            l0, l1 = CVB[j], CVB[j + 1]
            nl = l1 - l0
            mt = m_tiles[j]

            def batch_prod(eng, pool, tag, k, c0, c1, alloc=None):
                ncs = c1 - c0
                p = pool.tile([128, alloc or ncs, SB], BF16, tag=tag)
                mb = mt[:, k, None, :nl].broadcast_to((128, ncs, nl))
                eng.tensor_mul(
                    p[:, :ncs, :nl], mb, xtb[:, c0:c1, l0 + k : l0 + k + nl]
                )
                return p

            # pool (gpsimd) products: the c4/c5 pair first (their PE
            # streams run early), then per-c ascending. The tail block
            # keeps only the pair op on pool so the final drain is not
            # bounded by the slow engine; DVE (idle at the end otherwise)
            # takes the c0..3 k4..6 units as extra batches.
            tail = j == NCV - 1
            k5c3_pool = (not tail) and j in POOL_K5C3_BLOCKS
            prods = {}  # (k, c) -> [128, nl] slice
            pr = batch_prod(nc.gpsimd, p2_pool, "p2", 6, 4, 6)
            prods[(6, 4)], prods[(6, 5)] = pr[:, 0, :nl], pr[:, 1, :nl]
            if not tail:
                for c in range(4):
                    ks = POOL_KS[c]
                    if c == 3 and k5c3_pool:
                        ks = (5, 6)
                    for k in ks:
                        p = p1_pool.tile([128, SB], BF16, tag="p1")
                        nc.gpsimd.tensor_mul(
                            p[:, :nl],
                            mt[:, k, :nl],
                            xtb[:, c, l0 + k : l0 + k + nl],
                        )
                        prods[(k, c)] = p[:, :nl]

            # DVE batched products for the complement
            for k in range(4):
                p = batch_prod(nc.vector, p8_pool, "p8", k, 0, 8)
                for c in range(8):
                    prods[(k, c)] = p[:, c, :nl]
            p = batch_prod(nc.vector, p5_pool, "p5", 4, 3, 8)
            for c in range(3, 8):
                prods[(4, c)] = p[:, c - 3, :nl]
            if k5c3_pool:
                p = batch_prod(nc.vector, p5_pool, "p5", 5, 4, 8, alloc=5)
                for c in range(4, 8):
                    prods[(5, c)] = p[:, c - 4, :nl]
            else:
                p = batch_prod(nc.vector, p5_pool, "p5", 5, 3, 8)
                for c in range(3, 8):
                    prods[(5, c)] = p[:, c - 3, :nl]
            p = batch_prod(nc.vector, p2_pool, "p2", 6, 6, 8)
            prods[(6, 6)], prods[(6, 7)] = p[:, 0, :nl], p[:, 1, :nl]
            if tail:
                p = batch_prod(nc.vector, p5_pool, "p5", 6, 0, 4, alloc=5)
                for c in range(4):
                    prods[(6, c)] = p[:, c, :nl]
                pt = batch_prod(nc.vector, p5_pool, "p5", 4, 0, 3, alloc=5)
                for c in range(3):
                    prods[(4, c)] = pt[:, c, :nl]
                pt = batch_prod(nc.vector, p5_pool, "p5", 5, 0, 3, alloc=5)
                for c in range(3):
                    prods[(5, c)] = pt[:, c, :nl]

            at = acc_pool.tile([128, C, SB], BF16, tag="acc")
            acc_tiles[j] = at

            tree_cs = [c for c in (6, 7) if (j, c) in DVE_TREE]

            def stream(c, pcv, k0, k1):
                for k in range(k0, k1):
                    nc.tensor.matmul(
                        pcv[:, :nl],
                        identb_t,
                        prods[(k, c)],
                        start=(k == 0),
                        stop=(k == K - 1),
                    )
                if k1 == K:
                    nc.scalar.copy(at[:, c, :nl], pcv[:, :nl])

            if tail:
                # all products are DVE-made (plus the early pool pair), so
                # stream each c fully and store finished pairs immediately
                # to keep the final drain as short as possible
                for c in (6, 7, 4, 5, 0, 1, 2, 3):
                    pcv = p_cv.tile([128, SB], F32, tag="pcv")
                    stream(c, pcv, 0, K)
                    if c in (7, 5, 1, 3):
                        c0 = c - 1
                        nc.sync.dma_start(
                            out=out_v[:, c0 : c0 + 2, l0:l1],
                            in_=at[:, c0 : c0 + 2, :nl],
                        )
                return

            # c6/c7/c4/c5 stream their full k-sum (all their products come
            # from DVE batches or the early pool pair op); c0..3 open with
            # their DVE-made k0..3, then finish with pool-made k4..6 in
            # pool completion order so PE never parks on a late pool op.
            for c in [c for c in (6, 7) if c not in tree_cs] + [4, 5]:
                pcv = p_cv.tile([128, SB], F32, tag="pcv")
                stream(c, pcv, 0, K)
            open_cv = {}
            for c in range(4):
                open_cv[c] = p_cv.tile(
                    [128, SB], F32, tag="pcv", name=f"ocv{j}_{c}"
                )
                stream(c, open_cv[c], 0, 4)
            for c in range(4):
                stream(c, open_cv[c], 4, K)
            for c in tree_cs:
                ts = [prods[(k, c)] for k in range(K)]
                t01 = p2_pool.tile([128, 2, SB], BF16, tag="p2")
                a, bb = t01[:, 0, :nl], t01[:, 1, :nl]
                t23 = p2_pool.tile([128, 2, SB], BF16, tag="p2")
                cc, dd = t23[:, 0, :nl], t23[:, 1, :nl]
                nc.vector.tensor_add(a, ts[0], ts[1])
                nc.vector.tensor_add(bb, ts[2], ts[3])
                nc.vector.tensor_add(cc, ts[4], ts[5])
                nc.vector.tensor_add(dd, a, bb)
                nc.vector.tensor_add(a, cc, ts[6])
                nc.vector.tensor_add(at[:, c, :nl], dd, a)

        def store(j, split=1):
            l0, l1 = CVB[j], CVB[j + 1]
            nl = l1 - l0
            for i in range(split):
                c0, c1 = i * C // split, (i + 1) * C // split
                nc.sync.dma_start(
                    out=out_v[:, c0:c1, l0:l1],
                    in_=acc_tiles[j][:, c0:c1, :nl],
                )

        # ---- pipelined emission: fronts/mreps run 2 conv blocks ahead ----
        front(0)
        mrep(0)
        front(1)
        mrep(1)
        front(2)
        mrep(2)
        conv(0)
        store(0)
        front(3)
        mrep(3)
        conv(1)
        store(1)
        front(4)
        mrep(4)
        mrep(5)
        conv(2)
        store(2)
        conv(3)
        store(3)
        conv(4)
        store(4, split=2)
        conv(5)  # stores inline per c-pair

    nc.compile()
    return nc


_CACHE = {}


def _get_program():
    if "nc" not in _CACHE:
        _CACHE["nc"] = build_program()
    return _CACHE["nc"]


def kernel(x, W, b):
    x = np.asarray(x, dtype=np.float32)
    assert x.shape == (B, S, D), x.shape

    nc = _get_program()
    consts = _host_constants(W, b)
    in_maps = []
    for core in range(B):
        xt = np.ascontiguousarray(x[core].T).astype(ml_dtypes.bfloat16)
        xt = np.ascontiguousarray(
            xt.reshape(C, 128, S).transpose(1, 0, 2).reshape(128, C * S)
        )
        in_maps.append({"xt": xt, "consts": consts})
    res = bass_utils.run_bass_kernel_spmd(nc, in_maps, core_ids=list(range(B)))
    outs = []
    for core in range(B):
        arr = np.asarray(res.results[core]["out"]).reshape(128, C, L)
        outs.append(arr.transpose(2, 1, 0).reshape(L, D).astype(np.float32))
    return np.stack(outs, axis=0)


# revision 48
# speedup vs baseline: 1.0010x; 1.0010x over previous
"""Dynamic lightweight convolution TRN2 kernel.

out[b,l,d] = (1/K) * sum_k softmax_k(x[b,l+K-1,:] @ W + bias)[k, d%H] * x[b,l+k,d]

B=8, S=2048, D=1024, K=7, H=16, L=S-K+1=2042.
Sharding: data-parallel over batch, one batch element per NeuronCore (8 cores).

All sequence-major work stays in the transposed [d, s] layout end-to-end.
The host supplies x already transposed and bf16-cast (mirroring the
host-packed constants blob), and takes the output back in [d, l] bf16, so
the device program has NO transposes:
  1. DMA xtb[p, c, s] = x^T bf16 straight into SBUF.
  2. logits = W^T @ xtb on PE (fp32 PSUM accum over 8 d-chunks); E =
     exp(logits + bias) on ACT; selector matmul gives K*sum_k E (PE);
     Rinv ~ 1/that (DVE custom fast-reciprocal); en = E * Rinv (DVE).
  3. m[p, k, l] = en[16k + p%16, l+6] via 0/1 selector matmuls (PE) +
     ACT PSUM->SBUF bf16 copies, per conv block, emitted 2 blocks ahead
     of the conv so no engine ever starves waiting on weights.
  4. conv products p_k = m_k * x_(+k): DVE gets batched ops (one per
     k over a c-range, m broadcast over c via stride-0 dim, 2x bf16
     mode); GPSIMD (Pool) takes the (c, k) units in POOL_KS as per-c ops
     ordered c0->c3 so PE's per-c psum streams unblock progressively.
     The 7-way k-sum runs on PE as accumulating identity matmuls into
     PSUM (fp32), except DVE_TREE (block, c) units summed on DVE.
  5. ACT copies conv PSUM -> acc bf16; DMA out in [d, l] layout; host
     transposes back to [l, d] and upcasts to f32.
"""

import numpy as np
import ml_dtypes
from contextlib import ExitStack

import concourse.bacc as bacc
import concourse.tile as tile
from concourse import mybir
from concourse import bass_utils

K = 7
H = 16
B, S, D = 8, 2048, 1024
L = S - K + 1  # 2042
C = D // 128  # 8 d-chunks
SB = 512
KH = K * H  # 112

F32 = mybir.dt.float32
BF16 = mybir.dt.bfloat16

# front sub-ranges (logits/exp/denominator granularity)
FRONTS = [(0, 256), (256, 512), (512, 1024), (1024, 1536), (1536, 2048)]
# conv/mrep block boundaries; block j needs en/xtb cols < CVB[j+1] + 6
CVB = [0, 250, 506, 1018, 1530, 1914, 2042]
NCV = len(CVB) - 1

# Pool (gpsimd) product units: per-c k-tuples, processed c-ascending;
# plus one [c4,c5] pair op for k=6. DVE covers the complement with
# batched ops: k0..3 over c0..7, k4/k5 over c3..7, k6 over c6..7.
POOL_KS = {0: (4, 5, 6), 1: (4, 5, 6), 2: (4, 5, 6), 3: (6,)}
# blocks 0..3 also put (k5, c3) on pool to fill its early-finish idle
POOL_K5C3_BLOCKS = {0, 1, 2, 3}
# (block, c) units whose k-sum runs as a DVE add tree instead of PE psum
# accumulation (c=7 products are all DVE-made, so the tree is local).
DVE_TREE = {(2, 7), (3, 7), (4, 7)}

# byte offsets (per partition) inside the packed constants blob; split so
# the first DMA carries what front(0) needs (bias/W/selsum) and the
# second the rest (identity for PE streams, selk for mrep).
_OFF_BIAS = 0       # [112, 1] f32
_OFF_WT = 4         # [128, 8, 112] bf16
_OFF_SELSUM = 1796  # [112, 112] bf16
_SPLIT = 2020       # first-DMA byte count (505 f32 cols)
_OFF_IDENTB = 2020  # [128, 128] bf16
_OFF_SELK = 2276    # [112, 896] bf16
_CONST_BYTES = 4068  # 1017 f32 columns


def _host_constants(W, b):
    """Pack bias/W/selsum/identb/selk into one [128, 1017] f32 blob."""
    buf = np.zeros((128, _CONST_BYTES), np.uint8)

    def put(off, arr):
        by = np.ascontiguousarray(arr).view(np.uint8).reshape(arr.shape[0], -1)
        buf[: arr.shape[0], off : off + by.shape[1]] = by

    put(_OFF_BIAS, np.asarray(b, np.float32).reshape(KH, 1))
    # W [D, KH] -> [128, C, KH] chunks (d = c*128 + p)
    wt = np.asarray(W, np.float32).astype(ml_dtypes.bfloat16)
    wt = wt.reshape(C, 128, KH).transpose(1, 0, 2).reshape(128, C * KH)
    put(_OFF_WT, np.ascontiguousarray(wt))
    h = np.arange(KH) % H
    selsum = ((h[:, None] == h[None, :]) * float(K)).astype(ml_dtypes.bfloat16)
    put(_OFF_SELSUM, selsum)
    put(_OFF_IDENTB, np.eye(128).astype(ml_dtypes.bfloat16))
    selk = np.zeros((KH, K * 128), dtype=ml_dtypes.bfloat16)
    for k in range(K):
        for p in range(128):
            selk[16 * k + p % 16, k * 128 + p] = 1.0
    put(_OFF_SELK, selk)
    return buf.view(np.float32)


def build_program():
    nc = bacc.Bacc(
        "TRN2", target_bir_lowering=False, debug=False, enable_asserts=True
    )

    xt_d = nc.dram_tensor("xt", [128, C * S], BF16, kind="ExternalInput").ap()
    consts_d = nc.dram_tensor(
        "consts", [128, _CONST_BYTES // 4], F32, kind="ExternalInput"
    ).ap()
    out_d = nc.dram_tensor("out", [128, C * L], BF16, kind="ExternalOutput").ap()

    xt_v = xt_d.rearrange("p (c s) -> p c s", c=C)
    out_v = out_d.rearrange("p (c l) -> p c l", c=C)

    with tile.TileContext(nc) as tc, ExitStack() as ctx:
        singles = ctx.enter_context(tc.tile_pool(name="singles", bufs=1))
        m_pool = ctx.enter_context(tc.tile_pool(name="mw", bufs=3))
        acc_pool = ctx.enter_context(tc.tile_pool(name="acc", bufs=3))
        p8_pool = ctx.enter_context(tc.tile_pool(name="p8", bufs=6))
        p5_pool = ctx.enter_context(tc.tile_pool(name="p5", bufs=4))
        p2_pool = ctx.enter_context(tc.tile_pool(name="p2", bufs=8))
        p1_pool = ctx.enter_context(tc.tile_pool(name="p1", bufs=10))

        p_log = ctx.enter_context(tc.tile_pool(name="plog", bufs=1, space="PSUM"))
        p_mk = ctx.enter_context(tc.tile_pool(name="pmk", bufs=2, space="PSUM"))
        p_cv = ctx.enter_context(tc.tile_pool(name="pcv", bufs=5, space="PSUM"))

        # ---- constants (split: front deps first) + x blocks ----
        cblob = singles.tile([128, _CONST_BYTES // 4], F32)
        nc.sync.dma_start(
            out=cblob[:, : _SPLIT // 4], in_=consts_d[:, : _SPLIT // 4]
        )
        cbytes = cblob.bitcast(mybir.dt.uint8)

        def cview(off, nbytes, dt, rows=128):
            return cbytes[:rows, off : off + nbytes].bitcast(dt)

        bias_t = cview(_OFF_BIAS, 4, F32, rows=KH)
        wt = cview(_OFF_WT, 1792, BF16).rearrange("p (c n) -> p c n", c=C)
        selsum_t = cview(_OFF_SELSUM, 224, BF16, rows=KH)
        identb_t = cview(_OFF_IDENTB, 256, BF16)
        selk_t = cview(_OFF_SELK, 1792, BF16, rows=KH).rearrange(
            "c (k p) -> c k p", k=K
        )

        # ---- persistent tensors ----
        xtb = singles.tile([128, C, S], BF16)  # x^T bf16
        e_full = singles.tile([KH, S], BF16)  # exp(logits + b)
        rinv = singles.tile([KH, S], F32)  # 1 / (K * sum_k E)
        rinv_b = singles.tile([KH, S], BF16)  # bf16 copy (2x en-mult mode)
        en = singles.tile([KH, S], BF16)  # normalized kernel weights

        def load(s0, s1, eng=None):
            (eng or nc.sync).dma_start(
                out=xtb[:, :, s0:s1], in_=xt_v[:, :, s0:s1]
            )

        # first block load on the ACT hwdge queue so its descriptor-gen
        # setup overlaps the consts DMA's (transfers still serialize)
        load(0, 256, eng=nc.scalar)
        nc.sync.dma_start(
            out=cblob[:, _SPLIT // 4 :], in_=consts_d[:, _SPLIT // 4 :]
        )
        # GPSIMD ucode warmup: force the TT library load before real work
        warm = singles.tile([1, 8], BF16)
        nc.gpsimd.tensor_mul(warm, selsum_t[:1, :8], selsum_t[:1, :8])
        # Warmups run on a memset tile so they have NO DMA dependency and
        # fill the otherwise-idle window while x/consts DMAs are in flight:
        # ACT loads its Exp table, PE ramps its clock out of the cold
        # pstate so the first logits matmul runs at full speed.
        wtile = singles.tile([128, 128], BF16)
        nc.vector.memset(wtile, 1.0)
        warm_e = singles.tile([KH, 1], F32)
        nc.scalar.activation(
            warm_e, wtile[:KH, :1], mybir.ActivationFunctionType.Exp
        )
        wpsum = p_log.tile([KH, SB], F32, tag="plog")
        for _ in range(26):
            nc.tensor.matmul(
                wpsum[:, :128], wtile[:, :KH], wtile, start=True, stop=True
            )
        load(256, 512)
        load(512, 1024)
        load(1024, 1536)
        load(1536, 2048)

        def front(fi):
            """logits -> exp -> softmax denom -> normalized weights en."""
            s0, s1 = FRONTS[fi]
            sl = slice(s0, s1)
            ns = s1 - s0
            plog = p_log.tile([KH, SB], F32, tag="plog")
            for c in range(C):
                nc.tensor.matmul(
                    plog[:, :ns],
                    wt[:, c, :],
                    xtb[:, c, sl],
                    start=(c == 0),
                    stop=(c == C - 1),
                )
            nc.scalar.activation(
                e_full[:, sl],
                plog[:, :ns],
                mybir.ActivationFunctionType.Exp,
                bias=bias_t,
                scale=1.0,
            )
            psum = p_log.tile([KH, SB], F32, tag="plog")
            nc.tensor.matmul(
                psum[:, :ns], selsum_t, e_full[:, sl], start=True, stop=True
            )
            nc.vector.reciprocal_approx_fast(rinv[:, sl], psum[:, :ns])
            nc.scalar.copy(rinv_b[:, sl], rinv[:, sl])
            nc.vector.tensor_mul(en[:, sl], e_full[:, sl], rinv_b[:, sl])

        m_tiles = {}
        acc_tiles = {}

        def mrep(j):
            """m_j[p, k, i] = en[16k + p%16, CVB[j] + i + K - 1]."""
            l0, l1 = CVB[j], CVB[j + 1]
            nl = l1 - l0
            mt = m_pool.tile([128, K, SB], BF16, tag="mw")
            m_tiles[j] = mt
            # copy order serves consumers' first needs: DVE's k0 batch,
            # then pool's k6 pair / k4 / k5, then the remaining DVE batches
            for k in (0, 6, 4, 5, 1, 2, 3):
                pmk = p_mk.tile([128, SB], F32, tag="pmk")
                nc.tensor.matmul(
                    pmk[:, :nl],
                    selk_t[:, k, :],
                    en[:, l0 + K - 1 : l0 + K - 1 + nl],
                    start=True,
                    stop=True,
                )
                nc.scalar.copy(mt[:, k, :nl], pmk[:, :nl])

        def conv(j):
            l0, l1 = CVB[j], CVB[j + 1]
            nl = l1 - l0
            mt = m_tiles[j]

            def batch_prod(eng, pool, tag, k, c0, c1, alloc=None):
                ncs = c1 - c0
                p = pool.tile([128, alloc or ncs, SB], BF16, tag=tag)
                mb = mt[:, k, None, :nl].broadcast_to((128, ncs, nl))
                eng.tensor_mul(
                    p[:, :ncs, :nl], mb, xtb[:, c0:c1, l0 + k : l0 + k + nl]
                )
                return p

            # pool (gpsimd) products: the c4/c5 pair first (their PE
            # streams run early), then per-c ascending. The tail block
            # keeps only the pair op on pool so the final drain is not
            # bounded by the slow engine; DVE (idle at the end otherwise)
            # takes the c0..3 k4..6 units as extra batches.
            tail = j == NCV - 1
            k5c3_pool = (not tail) and j in POOL_K5C3_BLOCKS
            prods = {}  # (k, c) -> [128, nl] slice
            pr = batch_prod(nc.gpsimd, p2_pool, "p2", 6, 4, 6)
            prods[(6, 4)], prods[(6, 5)] = pr[:, 0, :nl], pr[:, 1, :nl]
            if not tail:
                for c in range(4):
                    ks = POOL_KS[c]
                    if c == 3 and k5c3_pool:
                        ks = (5, 6)
                    for k in ks:
                        p = p1_pool.tile([128, SB], BF16, tag="p1")
                        nc.gpsimd.tensor_mul(
                            p[:, :nl],
                            mt[:, k, :nl],
                            xtb[:, c, l0 + k : l0 + k + nl],
                        )
                        prods[(k, c)] = p[:, :nl]

            # DVE batched products for the complement
            for k in range(4):
                p = batch_prod(nc.vector, p8_pool, "p8", k, 0, 8)
                for c in range(8):
                    prods[(k, c)] = p[:, c, :nl]
            p = batch_prod(nc.vector, p5_pool, "p5", 4, 3, 8)
            for c in range(3, 8):
                prods[(4, c)] = p[:, c - 3, :nl]
            if k5c3_pool:
                p = batch_prod(nc.vector, p5_pool, "p5", 5, 4, 8, alloc=5)
                for c in range(4, 8):
                    prods[(5, c)] = p[:, c - 4, :nl]
            else:
                p = batch_prod(nc.vector, p5_pool, "p5", 5, 3, 8)
                for c in range(3, 8):
                    prods[(5, c)] = p[:, c - 3, :nl]
            p = batch_prod(nc.vector, p2_pool, "p2", 6, 6, 8)
            prods[(6, 6)], prods[(6, 7)] = p[:, 0, :nl], p[:, 1, :nl]
            if tail:
                p = batch_prod(nc.vector, p5_pool, "p5", 6, 0, 4, alloc=5)
                for c in range(4):
                    prods[(6, c)] = p[:, c, :nl]
                pt = batch_prod(nc.vector, p5_pool, "p5", 4, 0, 3, alloc=5)
                for c in range(3):
                    prods[(4, c)] = pt[:, c, :nl]
                pt = batch_prod(nc.vector, p5_pool, "p5", 5, 0, 3, alloc=5)
                for c in range(3):
                    prods[(5, c)] = pt[:, c, :nl]

            at = acc_pool.tile([128, C, SB], BF16, tag="acc")
            acc_tiles[j] = at

            tree_cs = [c for c in (6, 7) if (j, c) in DVE_TREE]

            def stream(c, pcv, k0, k1):
                for k in range(k0, k1):
                    nc.tensor.matmul(
                        pcv[:, :nl],
                        identb_t,
                        prods[(k, c)],
                        start=(k == 0),
                        stop=(k == K - 1),
                    )
                if k1 == K:
                    nc.scalar.copy(at[:, c, :nl], pcv[:, :nl])

            if tail:
                # all products are DVE-made (plus the early pool pair), so
                # stream each c fully and store finished pairs immediately
                # to keep the final drain as short as possible
                for c in (6, 7, 4, 5, 0, 1, 2, 3):
                    pcv = p_cv.tile([128, SB], F32, tag="pcv")
                    stream(c, pcv, 0, K)
                    if c in (7, 5, 1, 3):
                        c0 = c - 1
                        nc.sync.dma_start(
                            out=out_v[:, c0 : c0 + 2, l0:l1],
                            in_=at[:, c0 : c0 + 2, :nl],
                        )
                return

            # c6/c7/c4/c5 stream their full k-sum (all their products come
            # from DVE batches or the early pool pair op); c0..3 open with
            # their DVE-made k0..3, then finish with pool-made k4..6 in
            # pool completion order so PE never parks on a late pool op.
            for c in [c for c in (6, 7) if c not in tree_cs] + [4, 5]:
                pcv = p_cv.tile([128, SB], F32, tag="pcv")
                stream(c, pcv, 0, K)
            open_cv = {}
            for c in range(4):
                open_cv[c] = p_cv.tile(
                    [128, SB], F32, tag="pcv", name=f"ocv{j}_{c}"
                )
                stream(c, open_cv[c], 0, 4)
            for c in range(4):
                stream(c, open_cv[c], 4, K)
            for c in tree_cs:
                ts = [prods[(k, c)] for k in range(K)]
                t01 = p2_pool.tile([128, 2, SB], BF16, tag="p2")
                a, bb = t01[:, 0, :nl], t01[:, 1, :nl]
                t23 = p2_pool.tile([128, 2, SB], BF16, tag="p2")
                cc, dd = t23[:, 0, :nl], t23[:, 1, :nl]
                nc.vector.tensor_add(a, ts[0], ts[1])
                nc.vector.tensor_add(bb, ts[2], ts[3])
                nc.vector.tensor_add(cc, ts[4], ts[5])
                nc.vector.tensor_add(dd, a, bb)
                nc.vector.tensor_add(a, cc, ts[6])
                nc.vector.tensor_add(at[:, c, :nl], dd, a)

        def store(j, split=1):
            l0, l1 = CVB[j], CVB[j + 1]
            nl = l1 - l0
            for i in range(split):
                c0, c1 = i * C // split, (i + 1) * C // split
                nc.sync.dma_start(
                    out=out_v[:, c0:c1, l0:l1],
                    in_=acc_tiles[j][:, c0:c1, :nl],
                )

        # ---- pipelined emission: fronts/mreps run 2 conv blocks ahead ----
        front(0)
        mrep(0)
        front(1)
        mrep(1)
        front(2)
        mrep(2)
        conv(0)
        store(0)
        front(3)
        mrep(3)
        conv(1)
        store(1)
        front(4)
        mrep(4)
        mrep(5)
        conv(2)
        store(2)
        conv(3)
        store(3)
        conv(4)
        store(4, split=2)
        conv(5)  # stores inline per c-pair

    nc.compile()
    return nc


_CACHE = {}


def _get_program():
    if "nc" not in _CACHE:
        _CACHE["nc"] = build_program()
    return _CACHE["nc"]


def kernel(x, W, b):
    x = np.asarray(x, dtype=np.float32)
    assert x.shape == (B, S, D), x.shape

    nc = _get_program()
    consts = _host_constants(W, b)
    in_maps = []
    for core in range(B):
        xt = np.ascontiguousarray(x[core].T).astype(ml_dtypes.bfloat16)
        xt = np.ascontiguousarray(
            xt.reshape(C, 128, S).transpose(1, 0, 2).reshape(128, C * S)
        )
        in_maps.append({"xt": xt, "consts": consts})
    res = bass_utils.run_bass_kernel_spmd(nc, in_maps, core_ids=list(range(B)))
    outs = []
    for core in range(B):
        arr = np.asarray(res.results[core]["out"]).reshape(128, C, L)
        outs.append(arr.transpose(2, 1, 0).reshape(L, D).astype(np.float32))
    return np.stack(outs, axis=0)
